# revision 9
# baseline (speedup 1.0000x reference)
"""Trainium2 Bass kernel for nn_Decoder (LSTM decode loop, 20 steps).

Strategy:
- Data-parallel over batch: 4096 rows -> 8 cores x 512.
- Feature-major on-chip layout: activations are stored transposed
  [features(partitions), batch(free)], so the LSTM state h/c stays resident in
  SBUF across all 20 steps and no on-chip transposes are ever needed. All
  weights are pre-transposed on the host into lhsT tile layouts.
- Matmuls run in float32r (full PE rate; ~1e-4 relative rounding vs fp32).
- W_hh^T (16MB) and W_fcin^T (2MB) stay resident in SBUF; W_ih^T (8MB) is
  re-streamed from HBM every step in 8 per-hidden-tile chunks, double
  buffered. Biases are applied via the activation engine's free affine.
- The 2-class output softmax is computed exactly as sigmoid(l0 - l1) of the
  logit difference, so fcout reduces to a single dot-product row.
"""

import os
import numpy as np

import concourse.bass as bass
import concourse.mybir as mybir
import concourse.tile as tile
from concourse.bass_utils import run_bass_kernel_spmd
from concourse.vector_clock import ScopedClock

F32R = mybir.dt.float32r
F32 = mybir.dt.float32
AF = mybir.ActivationFunctionType
AX = mybir.AxisListType

N_CORES = 8
BS, HID, INP, NSPAN, NOUT, T = 4096, 1024, 512, 8, 2, 20
B = BS // N_CORES  # 512 per core
KH = HID // 128    # 8 hidden k-tiles
KI = INP // 128    # 4 input k-tiles
MG = 4 * HID // 128  # 32 gate m-tiles
MF = INP // 128    # 4 fcin m-tiles
KHR = 6            # W_hh k-tiles resident in SBUF; k=6,7 streamed per step
KS = KI + (KH - KHR)  # streamed lhsT tiles per (th, gate): 4 ih + 2 hh

# ---------------------------------------------------------------------------
# Walrus on this toolchain rejects >1 sync wait per instruction. Tile attaches
# several. Post-pass: move extra waits onto same-engine nops inserted before
# the owning instruction, and split the tail drain the same way.
_MAXW = 1


def _patched_drain_and_barrier(self, tick_clock, wait_clock):
    drain_inst = self.nc.sync.drain()
    wait_clock.add_sem_waits(drain_inst.ins, ScopedClock({None: tick_clock.global_clock}))
    si = drain_inst.ins.sync_info
    waits = list(si.on_wait) if si and si.on_wait else []
    if len(waits) > _MAXW:
        si.on_wait = waits[:_MAXW]
        rest = waits[_MAXW:]
        for i in range(0, len(rest), _MAXW):
            nop = self.nc.sync.nop(nofuse=True)
            nop.ins.sync_info = mybir.SyncInfo(on_wait=rest[i : i + _MAXW], on_update=[])
    self.nc.all_engine_barrier()
    assert self.sems is not None
    popped = self.nc._tile_sem_poison_stack.pop()
    assert popped is self._sem_poison
    self.nc.clear_and_free_semaphores(list(self.sems.allocated().values()))
    self.nc.all_engine_barrier()


tile.TileContext._drain_and_barrier = _patched_drain_and_barrier


def _split_multi_waits(nc):
    for fn in nc.m.functions:
        for bb in fn.blocks:
            new_insts = []
            changed = False
            for inst in bb.instructions:
                si = inst.sync_info
                waits = list(si.on_wait) if si and si.on_wait else []
                if len(waits) > _MAXW:
                    movable = [w for w in waits if w.wait_reg is None]
                    pinned = [w for w in waits if w.wait_reg is not None]
                    keep_n = max(0, _MAXW - len(pinned))
                    kept = pinned + movable[:keep_n]
                    for w in movable[keep_n:]:
                        nop = mybir.InstNoOp(
                            name=nc.get_next_instruction_name(),
                            engine=inst.engine,
                            sync_info=mybir.SyncInfo(on_wait=[w], on_update=[]),
                        )
                        nc.register_instruction(nop)
                        new_insts.append(nop)
                    si.on_wait = kept
                    changed = True
                new_insts.append(inst)
            if changed:
                bb.instructions = new_insts


# ---------------------------------------------------------------------------


def _build_program(n_steps=T, loop_iters=1):
    """Emit the full decode program for one core (SPMD across 8).

    loop_iters > 1 wraps the decode body in a hardware loop (timing use only).
    """
    nc = bass.Bass("TRN2", target_bir_lowering=False, debug=False)

    d = {}
    d["hx"] = nc.dram_tensor("hx", [HID, B], F32R, kind="ExternalInput").ap()
    d["cx"] = nc.dram_tensor("cx", [HID, B], F32, kind="ExternalInput").ap()
    d["whh"] = nc.dram_tensor("whh", [128, KHR * MG * 128], F32R, kind="ExternalInput").ap()
    d["wih"] = nc.dram_tensor("wih", [KH, 128, KS * 4 * 128], F32R, kind="ExternalInput").ap()
    d["wfc"] = nc.dram_tensor("wfc", [128, KH * MF * 128], F32R, kind="ExternalInput").ap()
    d["wsp"] = nc.dram_tensor("wsp", [128, KH * NSPAN], F32R, kind="ExternalInput").ap()
    d["wd"] = nc.dram_tensor("wd", [128, KH], F32R, kind="ExternalInput").ap()
    d["bg"] = nc.dram_tensor("bg", [128, MG], F32, kind="ExternalInput").ap()
    d["bfc"] = nc.dram_tensor("bfc", [128, MF], F32, kind="ExternalInput").ap()
    d["bsp"] = nc.dram_tensor("bsp", [1, NSPAN], F32R, kind="ExternalInput").ap()
    d["ones"] = nc.dram_tensor("ones", [1, 128], F32R, kind="ExternalInput").ap()
    d["bd"] = nc.dram_tensor("bd", [1, 1], F32, kind="ExternalInput").ap()
    d["nbd"] = nc.dram_tensor("nbd", [1, 1], F32, kind="ExternalInput").ap()

    nsp_d = nc.dram_tensor("nsp", [B, NSPAN], F32, kind="ExternalOutput").ap()
    outp_d = nc.dram_tensor("outp", [NOUT, n_steps, B], F32, kind="ExternalOutput").ap()

    with tile.TileContext(nc) as tc:
        with (
            tc.tile_pool(name="wres", bufs=1) as wres,      # resident weights
            tc.tile_pool(name="state", bufs=1) as state,    # h, c, inp
            tc.tile_pool(name="wstream", bufs=2) as wstream,  # W_ih chunks
            tc.tile_pool(name="small", bufs=2) as small,    # softmax/out staging
            tc.tile_pool(name="elw", bufs=2) as elw,        # elementwise temps
            tc.tile_pool(name="psum", bufs=8, space="PSUM") as ppool,
        ):
            # ---- resident weights and constants
            whh_t = wres.tile([128, KHR * MG * 128], F32R, name="whh_t")
            for k in range(KHR):
                nc.sync.dma_start(
                    whh_t[:, k * MG * 128 : (k + 1) * MG * 128],
                    d["whh"][:, k * MG * 128 : (k + 1) * MG * 128],
                )
            wfc_t = wres.tile([128, KH * MF * 128], F32R, name="wfc_t")
            nc.sync.dma_start(wfc_t[:], d["wfc"][:])
            wsp_t = wres.tile([128, KH * NSPAN], F32R, name="wsp_t")
            nc.sync.dma_start(wsp_t[:], d["wsp"][:])
            wd_t = wres.tile([128, KH], F32R, name="wd_t")
            nc.sync.dma_start(wd_t[:], d["wd"][:])
            bg_t = wres.tile([128, MG], F32, name="bg_t")
            nc.sync.dma_start(bg_t[:], d["bg"][:])
            bfc_t = wres.tile([128, MF], F32, name="bfc_t")
            nc.sync.dma_start(bfc_t[:], d["bfc"][:])
            bsp_t = wres.tile([1, NSPAN], F32R, name="bsp_t")
            nc.sync.dma_start(bsp_t[:], d["bsp"][:])
            ones_t = wres.tile([1, 128], F32R, name="ones_t")
            nc.sync.dma_start(ones_t[:], d["ones"][:])
            bd_t = wres.tile([1, 1], F32, name="bd_t")
            nc.sync.dma_start(bd_t[:], d["bd"][:])
            nbd_t = wres.tile([1, 1], F32, name="nbd_t")
            nc.sync.dma_start(nbd_t[:], d["nbd"][:])
            negone_t = wres.tile([1, 1], F32, name="negone_t")
            nc.gpsimd.memset(negone_t[:], -1.0)

            # ---- state (h double-buffered: read h_bufs[t%2], write h_bufs[(t+1)%2])
            h_bufs = [
                [state.tile([128, B], F32R, name=f"h{s_}_{k}") for k in range(KH)]
                for s_ in range(2)
            ]
            c = [state.tile([128, B], F32, name=f"c{k}") for k in range(KH)]
            inp = [state.tile([128, B], F32R, name=f"inp{k}") for k in range(KI)]
            h = h_bufs[0]
            for k in range(KH):
                nc.sync.dma_start(h[k][:], d["hx"][k * 128 : (k + 1) * 128, :])
                nc.sync.dma_start(c[k][:], d["cx"][k * 128 : (k + 1) * 128, :])

            # ---- num-span head on initial hidden state (batch-major)
            for bt in range(B // 128):
                ps_sp = ppool.tile([128, NSPAN], F32, tag="ps", name=f"ps_sp{bt}")
                for k in range(KH):
                    nc.tensor.matmul(
                        ps_sp[:],
                        h[k][:, bt * 128 : (bt + 1) * 128],
                        wsp_t[:, k * NSPAN : (k + 1) * NSPAN],
                        start=(k == 0),
                        stop=False,
                    )
                nc.tensor.matmul(ps_sp[:], ones_t[:], bsp_t[:], start=False, stop=True)
                negmax = small.tile([128, 1], F32, name=f"negmax{bt}", tag="negmax")
                nc.vector.reduce_max(negmax[:], ps_sp[:], axis=AX.X, negate=True)
                ex = small.tile([128, NSPAN], F32, name=f"ex{bt}", tag="ex")
                nc.scalar.activation(ex[:], ps_sp[:], AF.Exp, bias=negmax[:])
                ssum = small.tile([128, 1], F32, name=f"ssum{bt}", tag="ssum")
                nc.vector.reduce_sum(ssum[:], ex[:], axis=AX.X)
                rec = small.tile([128, 1], F32, name=f"rec{bt}", tag="rec")
                nc.vector.reciprocal(rec[:], ssum[:])
                nsp_o = small.tile([128, NSPAN], F32, name=f"nsp_o{bt}", tag="nsp_o")
                nc.vector.tensor_scalar_mul(nsp_o[:], ex[:], rec[:])
                nc.sync.dma_start(nsp_d[bt * 128 : (bt + 1) * 128, :], nsp_o[:])

            # ---- decode loop
            def decode_body():
                for t in range(n_steps):
                    first = t == 0 and loop_iters == 1
                    hr = h_bufs[t % 2]
                    hw = h_bufs[(t + 1) % 2]
                    for th in range(KH):
                        # stream this hidden tile's chunk: W_ih k0..3 (cols
                        # j=0..3) + W_hh k6,7 (cols j=4,5), per gate
                        wih_c = wstream.tile(
                            [128, KS * 4 * 128], F32R, tag="wih", name=f"wih_{t}_{th}"
                        )
                        nc.sync.dma_start(wih_c[:], d["wih"][th])
                        gps = []
                        for gi in range(4):
                            m = gi * KH + th
                            ps = ppool.tile([128, B], F32, tag="ps", name=f"ps_{t}_{th}_{gi}")
                            for k in range(KHR):
                                nc.tensor.matmul(
                                    ps[:],
                                    whh_t[:, (k * MG + m) * 128 : (k * MG + m + 1) * 128],
                                    hr[k][:],
                                    start=(k == 0),
                                    stop=False,
                                )
                            for k in range(KHR, KH):
                                j = KI + (k - KHR)
                                nc.tensor.matmul(
                                    ps[:],
                                    wih_c[:, (j * 4 + gi) * 128 : (j * 4 + gi + 1) * 128],
                                    hr[k][:],
                                    start=False,
                                    stop=(first and k == KH - 1),
                                )
                            if not first:
                                for k in range(KI):
                                    nc.tensor.matmul(
                                        ps[:],
                                        wih_c[:, (k * 4 + gi) * 128 : (k * 4 + gi + 1) * 128],
                                        inp[k][:],
                                        start=False,
                                        stop=(k == KI - 1),
                                    )
                            gps.append(ps)
                        ps_i, ps_f, ps_g, ps_o = gps
                        mi, mf_, mg_, mo = th, KH + th, 2 * KH + th, 3 * KH + th
                        # gate nonlinearities (DVE may read at most one PSUM
                        # operand per op, so sig(i) lands in SBUF)
                        t1 = elw.tile([128, B], F32, tag="t1", name=f"t1_{t}_{th}")
                        t2 = elw.tile([128, B], F32, tag="t2", name=f"t2_{t}_{th}")
                        nc.scalar.activation(t1[:], ps_i[:], AF.Sigmoid, bias=bg_t[:, mi : mi + 1])
                        nc.scalar.activation(ps_g[:], ps_g[:], AF.Tanh, bias=bg_t[:, mg_ : mg_ + 1])
                        nc.scalar.activation(ps_f[:], ps_f[:], AF.Sigmoid, bias=bg_t[:, mf_ : mf_ + 1])
                        nc.scalar.activation(ps_o[:], ps_o[:], AF.Sigmoid, bias=bg_t[:, mo : mo + 1])
                        # c' = sig(f)*c + sig(i)*tanh(g); h = sig(o)*tanh(c')
                        nc.vector.tensor_mul(t2[:], t1[:], ps_g[:])       # i*g -> SBUF
                        nc.vector.tensor_mul(ps_f[:], ps_f[:], c[th][:])  # f*c in place
                        nc.vector.tensor_add(c[th][:], t2[:], ps_f[:])
                        t3 = elw.tile([128, B], F32, tag="t1", name=f"t3_{t}_{th}")
                        nc.scalar.activation(t3[:], c[th][:], AF.Tanh)
                        nc.vector.tensor_mul(hw[th][:], ps_o[:], t3[:])

                    # fcin: inp = relu(W_fcin h + b)
                    for m in range(MF):
                        ps = ppool.tile([128, B], F32, tag="ps", name=f"psfc_{t}_{m}")
                        for k in range(KH):
                            nc.tensor.matmul(
                                ps[:],
                                wfc_t[:, (k * MF + m) * 128 : (k * MF + m + 1) * 128],
                                hw[k][:],
                                start=(k == 0),
                                stop=(k == KH - 1),
                            )
                        nc.scalar.activation(inp[m][:], ps[:], AF.Relu, bias=bfc_t[:, m : m + 1])

                    # fcout: d = wd . h ; p0 = sig(d + bd), p1 = sig(-d - bd)
                    ps_d = ppool.tile([1, B], F32, tag="ps", name=f"psd_{t}")
                    for k in range(KH):
                        nc.tensor.matmul(
                            ps_d[:], wd_t[:, k : k + 1], hw[k][:],
                            start=(k == 0), stop=(k == KH - 1),
                        )
                    ob0 = small.tile([1, B], F32, tag="ob0", name=f"ob0_{t}", bufs=1)
                    ob1 = small.tile([1, B], F32, tag="ob1", name=f"ob1_{t}", bufs=1)
                    nc.scalar.activation(ob0[:], ps_d[:], AF.Sigmoid, bias=bd_t[:])
                    nc.scalar.activation(
                        ob1[:], ps_d[:], AF.Sigmoid, bias=nbd_t[:], scale=negone_t[:]
                    )
                    nc.sync.dma_start(outp_d[0:1, t, :], ob0[:])
                    nc.sync.dma_start(outp_d[1:2, t, :], ob1[:])

            if loop_iters == 1:
                decode_body()
            else:
                with tc.For_i(0, loop_iters, 1):
                    decode_body()

            if os.environ.get("DEBUG_DUMP"):
                hd = nc.dram_tensor("h_dbg", [HID, B], F32R, kind="ExternalOutput").ap()
                cd = nc.dram_tensor("c_dbg", [HID, B], F32, kind="ExternalOutput").ap()
                ind = nc.dram_tensor("inp_dbg", [INP, B], F32R, kind="ExternalOutput").ap()
                hf = h_bufs[n_steps % 2]
                for k in range(KH):
                    nc.sync.dma_start(hd[k * 128 : (k + 1) * 128, :], hf[k][:])
                    nc.sync.dma_start(cd[k * 128 : (k + 1) * 128, :], c[k][:])
                for k in range(KI):
                    nc.sync.dma_start(ind[k * 128 : (k + 1) * 128, :], inp[k][:])

    _split_multi_waits(nc)
    return nc


_CACHE = {}


def _get_program(n_steps=T, loop_iters=1):
    key = (n_steps, loop_iters)
    if key not in _CACHE:
        _CACHE[key] = _build_program(n_steps, loop_iters)
    return _CACHE[key]


def _prep_inputs(hx, cx, W_ih, W_hh, b_ih, b_hh, W_fcin, b_fcin, W_fcout, b_fcout,
                 W_span, b_span):
    """Host-side: pre-transpose weights into lhsT tile layouts, shard batch."""
    f32 = np.float32
    whh_full = np.transpose(
        np.asarray(W_hh, f32).reshape(MG, 128, KH, 128), (3, 2, 0, 1)
    )  # [p, k, m, c]
    whh = np.ascontiguousarray(whh_full[:, :KHR]).reshape(128, KHR * MG * 128)
    # streamed chunk per hidden tile th: [p, j, gate, c] with j=0..3 -> W_ih
    # k-tiles, j=4..5 -> W_hh k-tiles 6,7 (for the 4 gate m-tiles of th)
    wih_p = np.transpose(
        np.asarray(W_ih, f32).reshape(4, KH, 128, KI, 128), (1, 0, 2, 3, 4)
    )  # [th, gate, c, k, p]
    wih = np.zeros((KH, 128, KS * 4 * 128), f32)
    for th in range(KH):
        for gi in range(4):
            for j in range(KS):
                colbase = (j * 4 + gi) * 128
                if j < KI:
                    # lhsT tile [p, c] = W_ih[(gi*KH+th)*128+c, j*128+p]
                    blk = wih_p[th, gi, :, j, :].T  # [p, c]
                else:
                    k = KHR + (j - KI)
                    m = gi * KH + th
                    blk = whh_full[:, k, m, :]  # [p, c]
                wih[th, :, colbase : colbase + 128] = blk
    wih = np.ascontiguousarray(wih)
    wfc = np.ascontiguousarray(
        np.transpose(np.asarray(W_fcin, f32).reshape(MF, 128, KH, 128), (3, 2, 0, 1))
    ).reshape(128, KH * MF * 128)
    wsp = np.ascontiguousarray(
        np.transpose(np.asarray(W_span, f32).reshape(NSPAN, KH, 128), (2, 1, 0))
    ).reshape(128, KH * NSPAN)
    wd_vec = np.asarray(W_fcout, f32)[0] - np.asarray(W_fcout, f32)[1]
    wd = np.ascontiguousarray(wd_vec.reshape(KH, 128).T)
    bg = np.ascontiguousarray(
        (np.asarray(b_ih, f32) + np.asarray(b_hh, f32)).reshape(MG, 128).T
    )
    bfc = np.ascontiguousarray(np.asarray(b_fcin, f32).reshape(MF, 128).T)
    bsp = np.asarray(b_span, f32).reshape(1, NSPAN)
    bd_v = f32(np.asarray(b_fcout, f32)[0] - np.asarray(b_fcout, f32)[1])
    ones = np.ones((1, 128), f32)

    shared = {
        "whh": whh, "wih": wih, "wfc": wfc, "wsp": wsp, "wd": wd,
        "bg": bg, "bfc": bfc, "bsp": bsp, "ones": ones,
        "bd": np.full((1, 1), bd_v, f32), "nbd": np.full((1, 1), -bd_v, f32),
    }
    hx = np.asarray(hx, f32)
    cx = np.asarray(cx, f32)
    maps = []
    for ci in range(N_CORES):
        sl = slice(ci * B, (ci + 1) * B)
        m = dict(shared)
        m["hx"] = np.ascontiguousarray(hx[sl].T)
        m["cx"] = np.ascontiguousarray(cx[sl].T)
        maps.append(m)
    return maps


def kernel(**inputs):
    maps = _prep_inputs(**inputs)
    nc = _get_program()
    res = run_bass_kernel_spmd(nc, maps, list(range(N_CORES)))
    num_spans = np.concatenate([res.results[i]["nsp"] for i in range(N_CORES)], axis=0)
    outputs = np.concatenate(
        [np.transpose(res.results[i]["outp"], (2, 1, 0)) for i in range(N_CORES)], axis=0
    )
    return num_spans, outputs
